# revision 47
# baseline (speedup 1.0000x reference)
"""Trainium2 Bass kernel for nn_ConvQuantizationWrapper.

The reference bit-slices an 8-bit quantized 3x3 conv into 32 (2-bit act x
1-bit weight) conv passes and recombines them with powers of two. That
decomposition exactly reconstructs

    out = conv2d(A, Wq) / (sa*sw) + bias
    A   = clip(round(x*sa - zp), 0, 255) + zp        (integers in [-128,127])
    Wq  = wrap_int8(round(w * sw))                   (integers in [-128,127])

The kernel runs one quantized conv, data-parallel over batch (8 images per
NeuronCore), with a 75%-PE-utilization mapping:

  - activations are quantized on DVE and PRE-SCALED by 1/(sa*sw) into bf16
    (pre-scaling lets the epilogue be a single add: out = dense+edge+bias)
  - per image pair (P,Q), outputs are computed per row-PAIR t
    (out rows 2t, 2t+1). Three buffers per pair:
      Xpair: [P ; Q]            (buffer row r = image row r-1)
      XP:    [P ; P shifted]    (high half: buffer row r = image row r)
      XQ:    [Q ; Q shifted]
    Four PSUM banks per chunk, 3 matmuls (kw taps) each:
      dP (dense, all 4 quadrants useful): low = P-even partial, hi = P-odd
      dQ (dense, parity flipped):         low = Q-odd,  hi = Q-even
      e0 (diag,  kh=0 edges):             low = P-even-edge, hi = Q-even-edge
      e1 (anti-diag, kh=2 edges):         low = Q-odd-edge,  hi = P-odd-edge
    -> 12 matmuls per 14 output rows x 2 images vs 18 for the block-diagonal
       9-tap scheme (33% less PE time); all matmul inputs remain exact.
  - combine on DVE/Pool: even rows (partition-aligned) go straight into the
    interleaved output tile Y; odd rows land in Yod and are moved across
    partition halves by SBUF->SBUF DMA.
  - outputs written to DRAM as bf16 (rel err ~1e-3, tolerance 2e-2), host
    upcasts to f32.
"""

import numpy as np
import ml_dtypes

import concourse.bacc as bacc
import concourse.mybir as mybir
import concourse.tile as tile
from concourse import bass_utils

N_CORES = 8
IMGS = 8          # images per core (batch 64 / 8 cores)
C = 64
H = W = 56
HP = 58           # padded rows
WP = 60           # padded row stride (image cols live at 2..57)
NPIX = H * W      # 3136
NT = H // 2       # 28 output row pairs
TCH = 7           # row pairs per chunk
V2CH = 8          # chunks per 9-tap (v2) pair
V2ROWS = 7        # rows per v2 chunk (psum shape matches v3 tiles)
NCHUNKS = NT // TCH
MAGIC = 12582912.0       # 1.5 * 2**23: float32 round-to-nearest trick

_nc_cache = {}

NDUM = 16         # PE warm-up matmuls (fill startup window, raise pstate)
NXB = 3           # X buffer slots


def _build(sa: float, zp: float, recip: float, reps: int = 1):
    key = (sa, zp, recip, reps)
    if key in _nc_cache:
        return _nc_cache[key]

    A = mybir.AluOpType
    nc = bacc.Bacc("TRN2", target_bir_lowering=False, debug=False)
    x_d = nc.dram_tensor("x", [IMGS, C, H, W], mybir.dt.float32,
                         kind="ExternalInput").ap()
    w_d = nc.dram_tensor("wt", [128, 21 * 128], mybir.dt.bfloat16,
                         kind="ExternalInput").ap()
    b_d = nc.dram_tensor("biasd", [128, 1], mybir.dt.float32,
                         kind="ExternalInput").ap()
    y_d = nc.dram_tensor("y", [IMGS, C, H, W], mybir.dt.bfloat16,
                         kind="ExternalOutput").ap()

    B1 = float(np.float32(MAGIC) - np.float32(zp))   # exact f32
    nB1 = -B1

    with tile.TileContext(nc) as tc:
        with (
            tc.tile_pool(name="const", bufs=1) as cpool,
            tc.tile_pool(name="xbuf", bufs=1) as xpool,
            tc.tile_pool(name="work", bufs=3) as wpool,
            tc.tile_pool(name="psum", bufs=2, space="PSUM") as ppool,
        ):
            w_sb = cpool.tile([128, 21 * 128], mybir.dt.bfloat16, name="w_sb")
            b_sb = cpool.tile([128, 1], mybir.dt.float32, name="b_sb")

            # persistent X buffers; interior rewritten per pair, borders
            # zeroed once here
            Xpairs, XPs, XQs = [], [], []
            for j in range(NXB):
                Xp = xpool.tile([128, HP, WP], mybir.dt.bfloat16,
                                name=f"Xp_{j}", tag=f"Xp_{j}")
                # compact dual-shift buffers: only odd buffer rows are ever
                # read by the dense passes -> store just those 28 rows
                XP = xpool.tile([128, NT, WP], mybir.dt.bfloat16,
                                name=f"XPd_{j}", tag=f"XPd_{j}")
                XQ = xpool.tile([128, NT, WP], mybir.dt.bfloat16,
                                name=f"XQd_{j}", tag=f"XQd_{j}")
                Xpairs.append(Xp); XPs.append(XP); XQs.append(XQ)

            def zero_borders(Xp, XP, XQ):
                # Xpair (both halves: buf row r = img row r-1); col borders
                # stay zero forever (TS2 writes cols 2:58 only); XP/XQ get
                # their borders via the full-width dup copies, only the rows
                # the dups never write need explicit zeroing.
                nc.vector.memset(Xp[:, 0:1, :].rearrange("p a b -> p (a b)"),
                                 0.0)
                nc.vector.memset(Xp[:, 57:58, :].rearrange("p a b -> p (a b)"),
                                 0.0)
                nc.vector.memset(Xp[:, 1:57, 0:2], 0.0)
                nc.vector.memset(Xp[:, 1:57, 58:60], 0.0)
                # XP/XQ need no border zeroing: all 28 compact rows are
                # real data and col borders arrive zero via the copies

            # PE warm-up
            wdum = cpool.tile([128, 128], mybir.dt.bfloat16, name="wdum")
            nc.vector.memset(wdum, 0.0)
            psd = ppool.tile([128, TCH, 56], mybir.dt.float32, name="psdum",
                             tag="e0")
            psdf = psd.rearrange("p t w -> p (t w)")
            for _ in range(NDUM):
                nc.tensor.matmul(psdf[:, 0:128], wdum, wdum,
                                 start=True, stop=True)

            zero_borders(Xpairs[0], XPs[0], XQs[0])

            def new_in_tiles():
                xf = wpool.tile([128, NPIX], mybir.dt.float32,
                                name="xf", tag="xf")
                t1 = wpool.tile([128, NPIX], mybir.dt.float32,
                                name="t1", tag="t1")
                return xf, t1

            def stage_dma_ts1(pair, xf, t1, rows):
                """Input DMA (sync ring) + TS1 (Pool) — engines that never
                touch PSUM, safe to issue ahead of compute."""
                for q in range(len(rows) - 1):
                    r0, r1 = rows[q], rows[q + 1]
                    e0_, e1_ = r0 * W, r1 * W
                    nc.sync.dma_start(
                        out=xf[:, e0_:e1_],
                        in_=x_d[2 * pair:2 * pair + 2, :, r0:r1].rearrange(
                            "i c h w -> (i c) (h w)"))
                    # t1 = x*sa + (MAGIC - zp)  -> MAGIC + q_in (rounded)
                    nc.gpsimd.tensor_scalar(
                        t1[:, e0_:e1_], xf[:, e0_:e1_],
                        sa, B1, op0=A.mult, op1=A.add)

            def stage_ts2(Xp, t1, r0, r1):
                """Abar = (t1 - (MAGIC-zp)) * recip on DVE (strided 3-D
                write: DVE — Pool is ~14x slower on these). Issued mid-chunk
                so DVE's in-order stream never head-of-line blocks on it."""
                e0_, e1_ = r0 * W, r1 * W
                nc.vector.tensor_scalar(
                    Xp[:, 1 + r0:1 + r1, 2:58],
                    t1[:, e0_:e1_].rearrange("p (h w) -> p h w", h=r1 - r0),
                    nB1, recip, op0=A.add, op1=A.mult)

            def stage_dup_slice(Xp, XP, XQ, r0, r1):
                """Build compact dual-shift buffers: XP[tau] = (P row 2tau ;
                P row 2tau+1), XQ likewise for Q. Same-partition halves via
                ACT copies; cross halves via SBUF->SBUF DMA. r0/r1 even."""
                X2 = Xp.rearrange("p (t two) w -> p t two w", two=2)
                a, b = r0 // 2, r1 // 2
                nc.scalar.copy(out=XP[0:64, a:b, :],     # P even (same)
                               in_=X2[0:64, a:b, 1, :])
                nc.scalar.copy(out=XQ[64:128, a:b, :],   # Q odd (same)
                               in_=X2[64:128, a + 1:b + 1, 0, :])
                nc.gpsimd.dma_start(                     # P odd (cross)
                    out=XP[64:128, a:b, :],
                    in_=X2[0:64, a + 1:b + 1, 0, :])
                nc.gpsimd.dma_start(                     # Q even (cross)
                    out=XQ[0:64, a:b, :],
                    in_=X2[64:128, a:b, 1, :])

            def stage_quant(pair, Xp, xf, t1, rows):
                stage_dma_ts1(pair, xf, t1, rows)
                for q in range(len(rows) - 1):
                    stage_ts2(Xp, t1, rows[q], rows[q + 1])

            def stage_dup(Xp, XP, XQ, rows):
                for q in range(len(rows) - 1):
                    stage_dup_slice(Xp, XP, XQ, rows[q], rows[q + 1])

            # weights+bias on the ACT HWDGE queue, parallel to input slices
            # v2 tap slots (12-20) first: pair 0 only needs those, so the
            # first real matmul isn't gated on the full 672KB weight bank
            nc.scalar.dma_start(out=w_sb[:, 12 * 128:], in_=w_d[:, 12 * 128:])
            nc.scalar.dma_start(out=b_sb, in_=b_d)
            nc.scalar.dma_start(out=w_sb[:, 0:12 * 128],
                                in_=w_d[:, 0:12 * 128])

            # hybrid schedule: pairs 0,3 run the 9-tap block-diagonal path
            # (no dup staging: fast head/tail, light DVE/ACT load); pairs
            # 1,2 run the 75%-util path (PE-light; its staging chain hides
            # under the longer v2 pairs)
            KIND = ['v2', 'v3', 'v3', 'v2']
            PTAGS = ['e0', 'e1', 'dP', 'dQ']

            P0ROWS = [0, 8, 16, 24, 32, 44, 56]
            xf0, t10 = new_in_tiles()
            stage_dma_ts1(0, xf0, t10, P0ROWS)
            for q in range(len(P0ROWS) - 1):
                stage_ts2(Xpairs[0], t10, P0ROWS[q], P0ROWS[q + 1])
            for j in range(1, NXB):
                zero_borders(Xpairs[j], XPs[j], XQs[j])
            # pair 1: DMA+TS1 up front (sync/Pool only); its DVE/ACT pieces
            # are deferred into pair 0's chunk loop so those in-order
            # engines don't head-of-line block on the staging chain
            xf1, t11 = new_in_tiles()
            stage_dma_ts1(1, xf1, t11, [0, 28, 56])

            seq = [(rep, pair) for rep in range(reps)
                   for pair in range(IMGS // 2)]
            pend = {1: t11}
            for si, (rep, pair) in enumerate(seq):
                j = pair % NXB
                Xp, XP, XQ = Xpairs[j], XPs[j], XQs[j]
                kind = KIND[pair]
                last_pair = (si == len(seq) - 1)

                # prefetch DMA+TS1 for pair si+2 (sync/Pool); DVE TS2 and
                # ACT/SWDGE dup pieces for pair si+1 go to chunk boundaries
                # so each lands on its engine only after its deps are ready
                defer = {c: [] for c in range(V2CH)}
                if si + 2 < len(seq):
                    np2 = seq[si + 2][1]
                    xf, t1 = new_in_tiles()
                    stage_dma_ts1(np2, xf, t1, [0, 28, 56])
                    pend[si + 2] = t1
                if si + 1 < len(seq):
                    np1 = seq[si + 1][1]
                    n1 = np1 % NXB
                    t1n = pend[si + 1]
                    nXp, nXP, nXQ = Xpairs[n1], XPs[n1], XQs[n1]
                    cA, cB, cC = (1, 2, 4) if kind == 'v2' else (1, 2, 3)
                    defer[cA].append(
                        lambda nXp=nXp, t1n=t1n: stage_ts2(nXp, t1n, 0, 28))
                    if KIND[np1] == 'v3':
                        defer[cB].append(
                            lambda nXp=nXp, nXP=nXP, nXQ=nXQ, t1n=t1n:
                            (stage_ts2(nXp, t1n, 28, 56),
                             stage_dup_slice(nXp, nXP, nXQ, 0, 28)))
                        defer[cC].append(
                            lambda nXp=nXp, nXP=nXP, nXQ=nXQ:
                            stage_dup_slice(nXp, nXP, nXQ, 28, 56))
                    else:
                        defer[cB].append(
                            lambda nXp=nXp, t1n=t1n:
                            stage_ts2(nXp, t1n, 28, 56))

                out_d = y_d[2 * pair:2 * pair + 2].rearrange(
                    "i c h w -> (i c) (h w)")

                if kind == 'v2':
                    # 9-tap block-diagonal pairs: psum = out - bias directly
                    # (activations are pre-scaled by recip)
                    Y2 = wpool.tile([128, NPIX], mybir.dt.bfloat16,
                                    name="Y2", tag="Y2")
                    for ch in range(V2CH):
                        ps = ppool.tile([128, V2ROWS, W], mybir.dt.float32,
                                        name="ps2", tag=PTAGS[ch % 4])
                        for t9 in range(9):
                            kh, kw = divmod(t9, 3)
                            rs = V2ROWS * ch + kh
                            cs = 1 + kw
                            nc.tensor.matmul(
                                ps,
                                w_sb[:, (12 + t9) * 128:(13 + t9) * 128],
                                Xp[:, rs:rs + V2ROWS, cs:cs + 56],
                                start=(t9 == 0), stop=(t9 == 8))
                        psf = ps.rearrange("p t w -> p (t w)")
                        if last_pair and ch == V2CH - 1:
                            # final chunk: halve epilogue/DMA granularity so
                            # the kernel-end critical path is shorter
                            base = ch * V2ROWS * W
                            half = V2ROWS * W // 2
                            for hh in range(2):
                                lo, hi = base + hh * half, base + (hh + 1) * half
                                nc.scalar.activation(
                                    out=Y2[:, lo:hi],
                                    in_=psf[:, hh * half:(hh + 1) * half],
                                    func=mybir.ActivationFunctionType.Identity,
                                    bias=b_sb, scale=1.0)
                                nc.scalar.dma_start(out=out_d[:, lo:hi],
                                                    in_=Y2[:, lo:hi])
                        else:
                            nc.scalar.activation(
                                out=Y2[:, ch * V2ROWS * W:
                                       (ch + 1) * V2ROWS * W],
                                in_=psf,
                                func=mybir.ActivationFunctionType.Identity,
                                bias=b_sb, scale=1.0)
                        if last_pair:
                            if ch == 5:
                                nc.scalar.dma_start(
                                    out=out_d[:, 0:6 * V2ROWS * W],
                                    in_=Y2[:, 0:6 * V2ROWS * W])
                            elif ch == 6:
                                lo = ch * V2ROWS * W
                                hi = (ch + 1) * V2ROWS * W
                                nc.scalar.dma_start(out=out_d[:, lo:hi],
                                                    in_=Y2[:, lo:hi])
                        elif ch == V2CH - 1:
                            nc.scalar.dma_start(out=out_d, in_=Y2)
                        for fn in defer[ch]:
                            fn()
                    continue

                Y = wpool.tile([128, NT, 2, W], mybir.dt.bfloat16,
                               name="Y", tag="Y")
                Yod = wpool.tile([128, NT, W], mybir.dt.bfloat16,
                                 name="Yod", tag="Yod")
                X2p = Xp.rearrange("p (t two) w -> p t two w", two=2)

                for ch in range(NCHUNKS):
                    t0 = ch * TCH
                    e0p = ppool.tile([128, TCH, W], mybir.dt.float32,
                                     name="e0p", tag="e0")
                    e1p = ppool.tile([128, TCH, W], mybir.dt.float32,
                                     name="e1p", tag="e1")
                    dPp = ppool.tile([128, TCH, W], mybir.dt.float32,
                                     name="dPp", tag="dP")
                    dQp = ppool.tile([128, TCH, W], mybir.dt.float32,
                                     name="dQp", tag="dQ")
                    for kw in range(3):
                        cs = 1 + kw
                        st, sp = (kw == 0), (kw == 2)
                        nc.tensor.matmul(
                            e0p, w_sb[:, (4 * kw + 2) * 128:
                                      (4 * kw + 3) * 128],
                            X2p[:, t0:t0 + TCH, 0, cs:cs + 56],
                            start=st, stop=sp)
                    for kw in range(3):
                        cs = 1 + kw
                        st, sp = (kw == 0), (kw == 2)
                        nc.tensor.matmul(
                            e1p, w_sb[:, (4 * kw + 3) * 128:
                                      (4 * kw + 4) * 128],
                            X2p[:, t0 + 1:t0 + TCH + 1, 1, cs:cs + 56],
                            start=st, stop=sp)
                    for kw in range(3):
                        cs = 1 + kw
                        st, sp = (kw == 0), (kw == 2)
                        nc.tensor.matmul(
                            dPp, w_sb[:, (4 * kw + 0) * 128:
                                      (4 * kw + 1) * 128],
                            XP[:, t0:t0 + TCH, cs:cs + 56],
                            start=st, stop=sp)
                    for kw in range(3):
                        cs = 1 + kw
                        st, sp = (kw == 0), (kw == 2)
                        nc.tensor.matmul(
                            dQp, w_sb[:, (4 * kw + 1) * 128:
                                      (4 * kw + 2) * 128],
                            XQ[:, t0:t0 + TCH, cs:cs + 56],
                            start=st, stop=sp)

                    # edge + bias -> SBUF (ACT; one PSUM operand per inst)
                    tmpE = wpool.tile([128, TCH, W], mybir.dt.bfloat16,
                                      name="tmpE", tag="tmpE")
                    tmpO = wpool.tile([128, TCH, W], mybir.dt.bfloat16,
                                      name="tmpO", tag="tmpO")
                    nc.scalar.activation(
                        out=tmpE, in_=e0p,
                        func=mybir.ActivationFunctionType.Identity,
                        bias=b_sb, scale=1.0)
                    nc.scalar.activation(
                        out=tmpO, in_=e1p,
                        func=mybir.ActivationFunctionType.Identity,
                        bias=b_sb, scale=1.0)

                    # combine: out = dense + (edge + bias)   (pre-scaled)
                    tsl = slice(t0, t0 + TCH)
                    nc.vector.scalar_tensor_tensor(
                        out=Y[0:64, tsl, 0, :], in0=dPp[0:64],
                        scalar=0.0, in1=tmpE[0:64],
                        op0=A.add, op1=A.add)
                    nc.vector.scalar_tensor_tensor(
                        out=Y[64:128, tsl, 0, :], in0=dQp[64:128],
                        scalar=0.0, in1=tmpE[64:128],
                        op0=A.add, op1=A.add)
                    nc.vector.scalar_tensor_tensor(
                        out=Yod[64:128, tsl, :], in0=dPp[64:128],
                        scalar=0.0, in1=tmpO[64:128],
                        op0=A.add, op1=A.add)
                    nc.vector.scalar_tensor_tensor(
                        out=Yod[0:64, tsl, :], in0=dQp[0:64],
                        scalar=0.0, in1=tmpO[0:64],
                        op0=A.add, op1=A.add)

                    # odd-row cross moves + output DMA
                    def cross(g0, g1):
                        # last pair on the ACT ring: the sync ring may still
                        # be draining and would delay the final output chain
                        eng = nc.scalar if last_pair else nc.sync
                        eng.dma_start(out=Y[0:64, g0:g1, 1, :],
                                      in_=Yod[64:128, g0:g1, :])
                        eng.dma_start(out=Y[64:128, g0:g1, 1, :],
                                      in_=Yod[0:64, g0:g1, :])

                    def dma_out(g0, g1):
                        nc.scalar.dma_start(
                            out=out_d[:, g0 * 2 * W:g1 * 2 * W],
                            in_=Y[:, g0:g1, :, :])

                    if not last_pair:
                        if ch == 1:
                            cross(0, 14)
                        elif ch == 3:
                            cross(14, 28)
                            dma_out(0, 28)
                    else:
                        if ch == 1:
                            cross(0, 14)
                            dma_out(0, 14)
                        elif ch == 2:
                            cross(14, 21)
                            dma_out(14, 21)
                        elif ch == 3:
                            cross(21, 28)
                            dma_out(21, 28)

                    # deferred staging pieces for upcoming pairs
                    for fn in defer[ch]:
                        fn()

    nc.compile()
    _nc_cache[key] = nc
    return nc


KH_dP = {(0, 0): 1, (1, 0): 2, (0, 1): 0, (1, 1): 1}  # g=0 even, g=1 odd
KH_dQ = {(0, 0): 0, (1, 0): 1, (0, 1): 1, (1, 1): 2}  # g=0 odd,  g=1 even


def _pack_weights(qw_eff):
    """qw_eff [o, i, 3, 3] -> [128, 21*128] lhsT bank (bf16):
    slots 0-11 = v3 (75%-util) banks, 12-20 = v2 block-diag 9-tap banks."""
    wt = np.zeros((128, 21, 128), np.float32)
    core = qw_eff.transpose(1, 2, 3, 0).reshape(C, 9, C)   # [c, kh*3+kw, o]
    for t9 in range(9):
        wt[:C, 12 + t9, :C] = core[:, t9, :]
        wt[C:, 12 + t9, C:] = core[:, t9, :]
    for kw in range(3):
        dP = np.zeros((128, 128), np.float32)
        dQ = np.zeros((128, 128), np.float32)
        e0 = np.zeros((128, 128), np.float32)
        e1 = np.zeros((128, 128), np.float32)
        for s in range(2):
            for g in range(2):
                dP[64 * s:64 * s + 64, 64 * g:64 * g + 64] = \
                    qw_eff[:, :, KH_dP[(s, g)], kw].T
                dQ[64 * s:64 * s + 64, 64 * g:64 * g + 64] = \
                    qw_eff[:, :, KH_dQ[(s, g)], kw].T
        e0[0:64, 0:64] = qw_eff[:, :, 0, kw].T
        e0[64:128, 64:128] = qw_eff[:, :, 0, kw].T
        e1[0:64, 64:128] = qw_eff[:, :, 2, kw].T
        e1[64:128, 0:64] = qw_eff[:, :, 2, kw].T
        wt[:, 4 * kw + 0] = dP
        wt[:, 4 * kw + 1] = dQ
        wt[:, 4 * kw + 2] = e0
        wt[:, 4 * kw + 3] = e1
    return np.ascontiguousarray(
        wt.reshape(128, 21 * 128)).astype(ml_dtypes.bfloat16)


def _prep(x, weight, bias, scale_a, scale_w, zero_point):
    x = np.ascontiguousarray(np.asarray(x, dtype=np.float32))
    weight = np.asarray(weight, dtype=np.float32)
    bias = np.asarray(bias, dtype=np.float32)
    sa = float(np.asarray(scale_a).reshape(-1)[0])
    sw = float(np.asarray(scale_w).reshape(-1)[0])
    zp = float(np.asarray(zero_point).reshape(-1)[0])

    # activation-clip guard: reference clips round(x*sa - zp) to [0, 255].
    # For in-range data the clip is a no-op; otherwise pre-clamp on host.
    amax = float(np.abs(x).max())
    if not (amax * abs(sa) < abs(zp if zp != 0 else 0) + 126.99 and
            -0.49 < -zp and sa * amax - zp < 255.49):
        f32 = np.float32
        lo = (f32(-0.49) + f32(zp)) / f32(sa)
        hi = (f32(255.49) + f32(zp)) / f32(sa)
        x = np.clip(x, lo, hi).astype(np.float32)

    qw = np.round(weight * np.float32(sw))
    qwi = qw.astype(np.int64)
    qw_eff = ((qwi + 128) % 256) - 128
    delta = qwi - qw_eff          # nonzero only if |qw| > 127

    wt_dup = _pack_weights(qw_eff.astype(np.float32))
    bias_dup = np.ascontiguousarray(
        np.concatenate([bias, bias])[:, None].astype(np.float32))

    sprod = np.float32(sw) * np.float32(sa)
    recip = float(np.float32(1.0) / sprod)

    corr = None
    if np.any(delta != 0):
        dsum = delta.sum(axis=1).astype(np.float64)  # [o, 3, 3]
        plane = np.zeros((C, H, W), np.float64)
        for kh in range(3):
            for kw in range(3):
                h0, h1 = max(0, 1 - kh), min(H, H + 1 - kh)
                w0, w1 = max(0, 1 - kw), min(W, W + 1 - kw)
                plane[:, h0:h1, w0:w1] += dsum[:, kh, kw][:, None, None]
        corr = (zp * plane * float(recip)).astype(np.float32)

    return x, wt_dup, bias_dup, sa, zp, recip, corr


def _run(x, weight, bias, scale_a, scale_w, zero_point, trace=False,
         tmpdir=None):
    x, wt_dup, bias_dup, sa, zp, recip, corr = _prep(
        x, weight, bias, scale_a, scale_w, zero_point)
    nc = _build(sa, zp, recip)
    n = x.shape[0]
    assert n == N_CORES * IMGS, f"expected batch {N_CORES * IMGS}, got {n}"
    in_maps = [
        {"x": np.ascontiguousarray(x[k * IMGS:(k + 1) * IMGS]),
         "wt": wt_dup, "biasd": bias_dup}
        for k in range(N_CORES)
    ]
    try:
        res = bass_utils.run_bass_kernel_spmd(
            nc, in_maps, core_ids=list(range(N_CORES)), trace=trace,
            tmpdir=tmpdir)
    except ModuleNotFoundError:
        res = bass_utils.run_bass_kernel_spmd(
            nc, in_maps, core_ids=list(range(N_CORES)), trace=False)
    y = np.concatenate([res.results[k]["y"] for k in range(N_CORES)], axis=0)
    y = y.astype(np.float32)
    if corr is not None:
        y = y + corr[None]
    return np.ascontiguousarray(y), res


def kernel(x, weight, bias, scale_a, scale_w, zero_point):
    y, _ = _run(x, weight, bias, scale_a, scale_w, zero_point, trace=False)
    return y


# revision 49
# speedup vs baseline: 1.0461x; 1.0461x over previous
"""Trainium2 Bass kernel for nn_ConvQuantizationWrapper.

The reference bit-slices an 8-bit quantized 3x3 conv into 32 (2-bit act x
1-bit weight) conv passes and recombines them with powers of two. That
decomposition exactly reconstructs

    out = conv2d(A, Wq) / (sa*sw) + bias
    A   = clip(round(x*sa - zp), 0, 255) + zp        (integers in [-128,127])
    Wq  = wrap_int8(round(w * sw))                   (integers in [-128,127])

The kernel runs one quantized conv, data-parallel over batch (8 images per
NeuronCore), with a 75%-PE-utilization mapping:

  - activations are quantized on DVE and PRE-SCALED by 1/(sa*sw) into bf16
    (pre-scaling lets the epilogue be a single add: out = dense+edge+bias)
  - per image pair (P,Q), outputs are computed per row-PAIR t
    (out rows 2t, 2t+1). Three buffers per pair:
      Xpair: [P ; Q]            (buffer row r = image row r-1)
      XP:    [P ; P shifted]    (high half: buffer row r = image row r)
      XQ:    [Q ; Q shifted]
    Four PSUM banks per chunk, 3 matmuls (kw taps) each:
      dP (dense, all 4 quadrants useful): low = P-even partial, hi = P-odd
      dQ (dense, parity flipped):         low = Q-odd,  hi = Q-even
      e0 (diag,  kh=0 edges):             low = P-even-edge, hi = Q-even-edge
      e1 (anti-diag, kh=2 edges):         low = Q-odd-edge,  hi = P-odd-edge
    -> 12 matmuls per 14 output rows x 2 images vs 18 for the block-diagonal
       9-tap scheme (33% less PE time); all matmul inputs remain exact.
  - combine on DVE/Pool: even rows (partition-aligned) go straight into the
    interleaved output tile Y; odd rows land in Yod and are moved across
    partition halves by SBUF->SBUF DMA.
  - outputs written to DRAM as bf16 (rel err ~1e-3, tolerance 2e-2), host
    upcasts to f32.
"""

import numpy as np
import ml_dtypes

import concourse.bacc as bacc
import concourse.mybir as mybir
import concourse.tile as tile
from concourse import bass_utils

N_CORES = 8
IMGS = 8          # images per core (batch 64 / 8 cores)
C = 64
H = W = 56
HP = 58           # padded rows
WP = 60           # padded row stride (image cols live at 2..57)
NPIX = H * W      # 3136
NT = H // 2       # 28 output row pairs
TCH = 7           # row pairs per chunk
V2CH = 8          # chunks per 9-tap (v2) pair
V2ROWS = 7        # rows per v2 chunk (psum shape matches v3 tiles)
NCHUNKS = NT // TCH
MAGIC = 12582912.0       # 1.5 * 2**23: float32 round-to-nearest trick

_nc_cache = {}

NDUM = 16         # PE warm-up matmuls (fill startup window, raise pstate)
NXB = 3           # X buffer slots


def _build(sa: float, zp: float, recip: float, reps: int = 1):
    key = (sa, zp, recip, reps)
    if key in _nc_cache:
        return _nc_cache[key]

    A = mybir.AluOpType
    nc = bacc.Bacc("TRN2", target_bir_lowering=False, debug=False)
    x_d = nc.dram_tensor("x", [IMGS, C, H, W], mybir.dt.float32,
                         kind="ExternalInput").ap()
    w_d = nc.dram_tensor("wt", [128, 21 * 128], mybir.dt.bfloat16,
                         kind="ExternalInput").ap()
    b_d = nc.dram_tensor("biasd", [128, 1], mybir.dt.float32,
                         kind="ExternalInput").ap()
    y_d = nc.dram_tensor("y", [IMGS, C, H, W], mybir.dt.bfloat16,
                         kind="ExternalOutput").ap()

    B1 = float(np.float32(MAGIC) - np.float32(zp))   # exact f32
    nB1 = -B1

    with tile.TileContext(nc) as tc:
        with (
            tc.tile_pool(name="const", bufs=1) as cpool,
            tc.tile_pool(name="xbuf", bufs=1) as xpool,
            tc.tile_pool(name="work", bufs=3) as wpool,
            tc.tile_pool(name="psum", bufs=2, space="PSUM") as ppool,
        ):
            w_sb = cpool.tile([128, 21 * 128], mybir.dt.bfloat16, name="w_sb")
            b_sb = cpool.tile([128, 1], mybir.dt.float32, name="b_sb")

            # persistent X buffers; interior rewritten per pair, borders
            # zeroed once here
            Xpairs, XPs, XQs = [], [], []
            for j in range(NXB):
                Xp = xpool.tile([128, HP, WP], mybir.dt.bfloat16,
                                name=f"Xp_{j}", tag=f"Xp_{j}")
                # compact dual-shift buffers: only odd buffer rows are ever
                # read by the dense passes -> store just those 28 rows
                XP = xpool.tile([128, NT, WP], mybir.dt.bfloat16,
                                name=f"XPd_{j}", tag=f"XPd_{j}")
                XQ = xpool.tile([128, NT, WP], mybir.dt.bfloat16,
                                name=f"XQd_{j}", tag=f"XQd_{j}")
                Xpairs.append(Xp); XPs.append(XP); XQs.append(XQ)

            def zero_borders(Xp, XP, XQ):
                # Xpair (both halves: buf row r = img row r-1); col borders
                # stay zero forever (TS2 writes cols 2:58 only); XP/XQ get
                # their borders via the full-width dup copies, only the rows
                # the dups never write need explicit zeroing.
                nc.vector.memset(Xp[:, 0:1, :].rearrange("p a b -> p (a b)"),
                                 0.0)
                nc.vector.memset(Xp[:, 57:58, :].rearrange("p a b -> p (a b)"),
                                 0.0)
                nc.vector.memset(Xp[:, 1:57, 0:2], 0.0)
                nc.vector.memset(Xp[:, 1:57, 58:60], 0.0)
                # XP/XQ need no border zeroing: all 28 compact rows are
                # real data and col borders arrive zero via the copies

            # PE warm-up
            wdum = cpool.tile([128, 128], mybir.dt.bfloat16, name="wdum")
            nc.vector.memset(wdum, 0.0)
            psd = ppool.tile([128, TCH, 56], mybir.dt.float32, name="psdum",
                             tag="e0")
            psdf = psd.rearrange("p t w -> p (t w)")
            for _ in range(NDUM):
                nc.tensor.matmul(psdf[:, 0:128], wdum, wdum,
                                 start=True, stop=True)

            zero_borders(Xpairs[0], XPs[0], XQs[0])

            def new_in_tiles():
                xf = wpool.tile([128, NPIX], mybir.dt.float32,
                                name="xf", tag="xf")
                t1 = wpool.tile([128, NPIX], mybir.dt.float32,
                                name="t1", tag="t1")
                return xf, t1

            def stage_dma_ts1(pair, xf, t1, rows):
                """Input DMA (sync ring) + TS1 (Pool) — engines that never
                touch PSUM, safe to issue ahead of compute."""
                for q in range(len(rows) - 1):
                    r0, r1 = rows[q], rows[q + 1]
                    e0_, e1_ = r0 * W, r1 * W
                    nc.sync.dma_start(
                        out=xf[:, e0_:e1_],
                        in_=x_d[2 * pair:2 * pair + 2, :, r0:r1].rearrange(
                            "i c h w -> (i c) (h w)"))
                    # t1 = x*sa + (MAGIC - zp)  -> MAGIC + q_in (rounded)
                    nc.gpsimd.tensor_scalar(
                        t1[:, e0_:e1_], xf[:, e0_:e1_],
                        sa, B1, op0=A.mult, op1=A.add)

            def stage_ts2(Xp, t1, r0, r1):
                """Abar = (t1 - (MAGIC-zp)) * recip on DVE (strided 3-D
                write: DVE — Pool is ~14x slower on these). Issued mid-chunk
                so DVE's in-order stream never head-of-line blocks on it."""
                e0_, e1_ = r0 * W, r1 * W
                nc.vector.tensor_scalar(
                    Xp[:, 1 + r0:1 + r1, 2:58],
                    t1[:, e0_:e1_].rearrange("p (h w) -> p h w", h=r1 - r0),
                    nB1, recip, op0=A.add, op1=A.mult)

            def stage_dup_slice(Xp, XP, XQ, r0, r1):
                """Build compact dual-shift buffers: XP[tau] = (P row 2tau ;
                P row 2tau+1), XQ likewise for Q. Same-partition halves via
                ACT copies; cross halves via SBUF->SBUF DMA. r0/r1 even."""
                X2 = Xp.rearrange("p (t two) w -> p t two w", two=2)
                a, b = r0 // 2, r1 // 2
                nc.scalar.copy(out=XP[0:64, a:b, :],     # P even (same)
                               in_=X2[0:64, a:b, 1, :])
                nc.scalar.copy(out=XQ[64:128, a:b, :],   # Q odd (same)
                               in_=X2[64:128, a + 1:b + 1, 0, :])
                nc.gpsimd.dma_start(                     # P odd (cross)
                    out=XP[64:128, a:b, :],
                    in_=X2[0:64, a + 1:b + 1, 0, :])
                nc.gpsimd.dma_start(                     # Q even (cross)
                    out=XQ[0:64, a:b, :],
                    in_=X2[64:128, a:b, 1, :])

            def stage_quant(pair, Xp, xf, t1, rows):
                stage_dma_ts1(pair, xf, t1, rows)
                for q in range(len(rows) - 1):
                    stage_ts2(Xp, t1, rows[q], rows[q + 1])

            def stage_dup(Xp, XP, XQ, rows):
                for q in range(len(rows) - 1):
                    stage_dup_slice(Xp, XP, XQ, rows[q], rows[q + 1])

            # weights+bias on the ACT HWDGE queue, parallel to input slices
            # weights in two PARALLEL transfers: pair-0's 9-tap slots on the
            # ACT ring, the v3 banks on the separate SWDGE ring (needed only
            # from pair 1, ~21us in) — first real matmul gates ~1.5us sooner
            nc.scalar.dma_start(out=w_sb[:, 12 * 128:], in_=w_d[:, 12 * 128:])
            nc.scalar.dma_start(out=b_sb, in_=b_d)
            nc.gpsimd.dma_start(out=w_sb[:, 0:12 * 128],
                                in_=w_d[:, 0:12 * 128])

            # hybrid schedule: pairs 0,3 run the 9-tap block-diagonal path
            # (no dup staging: fast head/tail, light DVE/ACT load); pairs
            # 1,2 run the 75%-util path (PE-light; its staging chain hides
            # under the longer v2 pairs)
            KIND = ['v2', 'v3', 'v3', 'v2']
            PTAGS = ['e0', 'e1', 'dP', 'dQ']

            P0ROWS = [0, 8, 16, 24, 32, 44, 56]
            xf0, t10 = new_in_tiles()
            stage_dma_ts1(0, xf0, t10, P0ROWS)
            for q in range(len(P0ROWS) - 1):
                stage_ts2(Xpairs[0], t10, P0ROWS[q], P0ROWS[q + 1])
            for j in range(1, NXB):
                zero_borders(Xpairs[j], XPs[j], XQs[j])
            # pair 1: DMA+TS1 up front (sync/Pool only); its DVE/ACT pieces
            # are deferred into pair 0's chunk loop so those in-order
            # engines don't head-of-line block on the staging chain
            xf1, t11 = new_in_tiles()
            stage_dma_ts1(1, xf1, t11, [0, 28, 56])

            seq = [(rep, pair) for rep in range(reps)
                   for pair in range(IMGS // 2)]
            pend = {1: t11}
            for si, (rep, pair) in enumerate(seq):
                j = pair % NXB
                Xp, XP, XQ = Xpairs[j], XPs[j], XQs[j]
                kind = KIND[pair]
                last_pair = (si == len(seq) - 1)

                # prefetch DMA+TS1 for pair si+2 (sync/Pool); DVE TS2 and
                # ACT/SWDGE dup pieces for pair si+1 go to chunk boundaries
                # so each lands on its engine only after its deps are ready
                defer = {c: [] for c in range(V2CH)}
                if si + 2 < len(seq):
                    np2 = seq[si + 2][1]
                    xf, t1 = new_in_tiles()
                    stage_dma_ts1(np2, xf, t1, [0, 28, 56])
                    pend[si + 2] = t1
                if si + 1 < len(seq):
                    np1 = seq[si + 1][1]
                    n1 = np1 % NXB
                    t1n = pend[si + 1]
                    nXp, nXP, nXQ = Xpairs[n1], XPs[n1], XQs[n1]
                    cA, cB, cC = (1, 2, 4) if kind == 'v2' else (1, 2, 3)
                    defer[cA].append(
                        lambda nXp=nXp, t1n=t1n: stage_ts2(nXp, t1n, 0, 28))
                    if KIND[np1] == 'v3':
                        defer[cB].append(
                            lambda nXp=nXp, nXP=nXP, nXQ=nXQ, t1n=t1n:
                            (stage_ts2(nXp, t1n, 28, 56),
                             stage_dup_slice(nXp, nXP, nXQ, 0, 28)))
                        defer[cC].append(
                            lambda nXp=nXp, nXP=nXP, nXQ=nXQ:
                            stage_dup_slice(nXp, nXP, nXQ, 28, 56))
                    else:
                        defer[cB].append(
                            lambda nXp=nXp, t1n=t1n:
                            stage_ts2(nXp, t1n, 28, 56))

                out_d = y_d[2 * pair:2 * pair + 2].rearrange(
                    "i c h w -> (i c) (h w)")

                if kind == 'v2':
                    # 9-tap block-diagonal pairs: psum = out - bias directly
                    # (activations are pre-scaled by recip)
                    Y2 = wpool.tile([128, NPIX], mybir.dt.bfloat16,
                                    name="Y2", tag="Y2")
                    for ch in range(V2CH):
                        ps = ppool.tile([128, V2ROWS, W], mybir.dt.float32,
                                        name="ps2", tag=PTAGS[ch % 4])
                        for t9 in range(9):
                            kh, kw = divmod(t9, 3)
                            rs = V2ROWS * ch + kh
                            cs = 1 + kw
                            nc.tensor.matmul(
                                ps,
                                w_sb[:, (12 + t9) * 128:(13 + t9) * 128],
                                Xp[:, rs:rs + V2ROWS, cs:cs + 56],
                                start=(t9 == 0), stop=(t9 == 8))
                        nc.scalar.activation(
                            out=Y2[:, ch * V2ROWS * W:(ch + 1) * V2ROWS * W],
                            in_=ps.rearrange("p t w -> p (t w)"),
                            func=mybir.ActivationFunctionType.Identity,
                            bias=b_sb, scale=1.0)
                        if last_pair:
                            if ch == 5:
                                nc.scalar.dma_start(
                                    out=out_d[:, 0:6 * V2ROWS * W],
                                    in_=Y2[:, 0:6 * V2ROWS * W])
                            elif ch >= 6:
                                lo = ch * V2ROWS * W
                                hi = (ch + 1) * V2ROWS * W
                                nc.scalar.dma_start(out=out_d[:, lo:hi],
                                                    in_=Y2[:, lo:hi])
                        elif ch == V2CH - 1:
                            nc.scalar.dma_start(out=out_d, in_=Y2)
                        for fn in defer[ch]:
                            fn()
                    continue

                Y = wpool.tile([128, NT, 2, W], mybir.dt.bfloat16,
                               name="Y", tag="Y")
                Yod = wpool.tile([128, NT, W], mybir.dt.bfloat16,
                                 name="Yod", tag="Yod")
                X2p = Xp.rearrange("p (t two) w -> p t two w", two=2)

                for ch in range(NCHUNKS):
                    t0 = ch * TCH
                    e0p = ppool.tile([128, TCH, W], mybir.dt.float32,
                                     name="e0p", tag="e0")
                    e1p = ppool.tile([128, TCH, W], mybir.dt.float32,
                                     name="e1p", tag="e1")
                    dPp = ppool.tile([128, TCH, W], mybir.dt.float32,
                                     name="dPp", tag="dP")
                    dQp = ppool.tile([128, TCH, W], mybir.dt.float32,
                                     name="dQp", tag="dQ")
                    for kw in range(3):
                        cs = 1 + kw
                        st, sp = (kw == 0), (kw == 2)
                        nc.tensor.matmul(
                            e0p, w_sb[:, (4 * kw + 2) * 128:
                                      (4 * kw + 3) * 128],
                            X2p[:, t0:t0 + TCH, 0, cs:cs + 56],
                            start=st, stop=sp)
                    for kw in range(3):
                        cs = 1 + kw
                        st, sp = (kw == 0), (kw == 2)
                        nc.tensor.matmul(
                            e1p, w_sb[:, (4 * kw + 3) * 128:
                                      (4 * kw + 4) * 128],
                            X2p[:, t0 + 1:t0 + TCH + 1, 1, cs:cs + 56],
                            start=st, stop=sp)
                    for kw in range(3):
                        cs = 1 + kw
                        st, sp = (kw == 0), (kw == 2)
                        nc.tensor.matmul(
                            dPp, w_sb[:, (4 * kw + 0) * 128:
                                      (4 * kw + 1) * 128],
                            XP[:, t0:t0 + TCH, cs:cs + 56],
                            start=st, stop=sp)
                    for kw in range(3):
                        cs = 1 + kw
                        st, sp = (kw == 0), (kw == 2)
                        nc.tensor.matmul(
                            dQp, w_sb[:, (4 * kw + 1) * 128:
                                      (4 * kw + 2) * 128],
                            XQ[:, t0:t0 + TCH, cs:cs + 56],
                            start=st, stop=sp)

                    # edge + bias -> SBUF (ACT; one PSUM operand per inst)
                    tmpE = wpool.tile([128, TCH, W], mybir.dt.bfloat16,
                                      name="tmpE", tag="tmpE")
                    tmpO = wpool.tile([128, TCH, W], mybir.dt.bfloat16,
                                      name="tmpO", tag="tmpO")
                    nc.scalar.activation(
                        out=tmpE, in_=e0p,
                        func=mybir.ActivationFunctionType.Identity,
                        bias=b_sb, scale=1.0)
                    nc.scalar.activation(
                        out=tmpO, in_=e1p,
                        func=mybir.ActivationFunctionType.Identity,
                        bias=b_sb, scale=1.0)

                    # combine: out = dense + (edge + bias)   (pre-scaled)
                    tsl = slice(t0, t0 + TCH)
                    nc.vector.scalar_tensor_tensor(
                        out=Y[0:64, tsl, 0, :], in0=dPp[0:64],
                        scalar=0.0, in1=tmpE[0:64],
                        op0=A.add, op1=A.add)
                    nc.vector.scalar_tensor_tensor(
                        out=Y[64:128, tsl, 0, :], in0=dQp[64:128],
                        scalar=0.0, in1=tmpE[64:128],
                        op0=A.add, op1=A.add)
                    nc.vector.scalar_tensor_tensor(
                        out=Yod[64:128, tsl, :], in0=dPp[64:128],
                        scalar=0.0, in1=tmpO[64:128],
                        op0=A.add, op1=A.add)
                    nc.vector.scalar_tensor_tensor(
                        out=Yod[0:64, tsl, :], in0=dQp[0:64],
                        scalar=0.0, in1=tmpO[0:64],
                        op0=A.add, op1=A.add)

                    # odd-row cross moves + output DMA
                    def cross(g0, g1):
                        # last pair on the ACT ring: the sync ring may still
                        # be draining and would delay the final output chain
                        eng = nc.scalar if last_pair else nc.sync
                        eng.dma_start(out=Y[0:64, g0:g1, 1, :],
                                      in_=Yod[64:128, g0:g1, :])
                        eng.dma_start(out=Y[64:128, g0:g1, 1, :],
                                      in_=Yod[0:64, g0:g1, :])

                    def dma_out(g0, g1):
                        nc.scalar.dma_start(
                            out=out_d[:, g0 * 2 * W:g1 * 2 * W],
                            in_=Y[:, g0:g1, :, :])

                    if not last_pair:
                        if ch == 1:
                            cross(0, 14)
                        elif ch == 3:
                            cross(14, 28)
                            dma_out(0, 28)
                    else:
                        if ch == 1:
                            cross(0, 14)
                            dma_out(0, 14)
                        elif ch == 2:
                            cross(14, 21)
                            dma_out(14, 21)
                        elif ch == 3:
                            cross(21, 28)
                            dma_out(21, 28)

                    # deferred staging pieces for upcoming pairs
                    for fn in defer[ch]:
                        fn()

    nc.compile()
    _nc_cache[key] = nc
    return nc


KH_dP = {(0, 0): 1, (1, 0): 2, (0, 1): 0, (1, 1): 1}  # g=0 even, g=1 odd
KH_dQ = {(0, 0): 0, (1, 0): 1, (0, 1): 1, (1, 1): 2}  # g=0 odd,  g=1 even


def _pack_weights(qw_eff):
    """qw_eff [o, i, 3, 3] -> [128, 21*128] lhsT bank (bf16):
    slots 0-11 = v3 (75%-util) banks, 12-20 = v2 block-diag 9-tap banks."""
    wt = np.zeros((128, 21, 128), np.float32)
    core = qw_eff.transpose(1, 2, 3, 0).reshape(C, 9, C)   # [c, kh*3+kw, o]
    for t9 in range(9):
        wt[:C, 12 + t9, :C] = core[:, t9, :]
        wt[C:, 12 + t9, C:] = core[:, t9, :]
    for kw in range(3):
        dP = np.zeros((128, 128), np.float32)
        dQ = np.zeros((128, 128), np.float32)
        e0 = np.zeros((128, 128), np.float32)
        e1 = np.zeros((128, 128), np.float32)
        for s in range(2):
            for g in range(2):
                dP[64 * s:64 * s + 64, 64 * g:64 * g + 64] = \
                    qw_eff[:, :, KH_dP[(s, g)], kw].T
                dQ[64 * s:64 * s + 64, 64 * g:64 * g + 64] = \
                    qw_eff[:, :, KH_dQ[(s, g)], kw].T
        e0[0:64, 0:64] = qw_eff[:, :, 0, kw].T
        e0[64:128, 64:128] = qw_eff[:, :, 0, kw].T
        e1[0:64, 64:128] = qw_eff[:, :, 2, kw].T
        e1[64:128, 0:64] = qw_eff[:, :, 2, kw].T
        wt[:, 4 * kw + 0] = dP
        wt[:, 4 * kw + 1] = dQ
        wt[:, 4 * kw + 2] = e0
        wt[:, 4 * kw + 3] = e1
    return np.ascontiguousarray(
        wt.reshape(128, 21 * 128)).astype(ml_dtypes.bfloat16)


def _prep(x, weight, bias, scale_a, scale_w, zero_point):
    x = np.ascontiguousarray(np.asarray(x, dtype=np.float32))
    weight = np.asarray(weight, dtype=np.float32)
    bias = np.asarray(bias, dtype=np.float32)
    sa = float(np.asarray(scale_a).reshape(-1)[0])
    sw = float(np.asarray(scale_w).reshape(-1)[0])
    zp = float(np.asarray(zero_point).reshape(-1)[0])

    # activation-clip guard: reference clips round(x*sa - zp) to [0, 255].
    # For in-range data the clip is a no-op; otherwise pre-clamp on host.
    amax = float(np.abs(x).max())
    if not (amax * abs(sa) < abs(zp if zp != 0 else 0) + 126.99 and
            -0.49 < -zp and sa * amax - zp < 255.49):
        f32 = np.float32
        lo = (f32(-0.49) + f32(zp)) / f32(sa)
        hi = (f32(255.49) + f32(zp)) / f32(sa)
        x = np.clip(x, lo, hi).astype(np.float32)

    qw = np.round(weight * np.float32(sw))
    qwi = qw.astype(np.int64)
    qw_eff = ((qwi + 128) % 256) - 128
    delta = qwi - qw_eff          # nonzero only if |qw| > 127

    wt_dup = _pack_weights(qw_eff.astype(np.float32))
    bias_dup = np.ascontiguousarray(
        np.concatenate([bias, bias])[:, None].astype(np.float32))

    sprod = np.float32(sw) * np.float32(sa)
    recip = float(np.float32(1.0) / sprod)

    corr = None
    if np.any(delta != 0):
        dsum = delta.sum(axis=1).astype(np.float64)  # [o, 3, 3]
        plane = np.zeros((C, H, W), np.float64)
        for kh in range(3):
            for kw in range(3):
                h0, h1 = max(0, 1 - kh), min(H, H + 1 - kh)
                w0, w1 = max(0, 1 - kw), min(W, W + 1 - kw)
                plane[:, h0:h1, w0:w1] += dsum[:, kh, kw][:, None, None]
        corr = (zp * plane * float(recip)).astype(np.float32)

    return x, wt_dup, bias_dup, sa, zp, recip, corr


def _run(x, weight, bias, scale_a, scale_w, zero_point, trace=False,
         tmpdir=None):
    x, wt_dup, bias_dup, sa, zp, recip, corr = _prep(
        x, weight, bias, scale_a, scale_w, zero_point)
    nc = _build(sa, zp, recip)
    n = x.shape[0]
    assert n == N_CORES * IMGS, f"expected batch {N_CORES * IMGS}, got {n}"
    in_maps = [
        {"x": np.ascontiguousarray(x[k * IMGS:(k + 1) * IMGS]),
         "wt": wt_dup, "biasd": bias_dup}
        for k in range(N_CORES)
    ]
    try:
        res = bass_utils.run_bass_kernel_spmd(
            nc, in_maps, core_ids=list(range(N_CORES)), trace=trace,
            tmpdir=tmpdir)
    except ModuleNotFoundError:
        res = bass_utils.run_bass_kernel_spmd(
            nc, in_maps, core_ids=list(range(N_CORES)), trace=False)
    y = np.concatenate([res.results[k]["y"] for k in range(N_CORES)], axis=0)
    y = y.astype(np.float32)
    if corr is not None:
        y = y + corr[None]
    return np.ascontiguousarray(y), res


def kernel(x, weight, bias, scale_a, scale_w, zero_point):
    y, _ = _run(x, weight, bias, scale_a, scale_w, zero_point, trace=False)
    return y
